# revision 1
# baseline (speedup 1.0000x reference)
"""Bottom-Up Hidden Tree Markov Model upward pass on 8 Trainium2 NeuronCores.

Problem: complete 8-ary forest (2 trees x 299593 nodes, depth 6), C=8 hidden
states, 32 symbols, 16 independent generative models. Output: per-tree
log-likelihood (2, 16).

Sharding: core = (tree, quarter-of-tree). Each core runs the full upward pass
over its quarter (2 complete depth-1 subtrees): 65536 leaves -> 8192 -> 1024
-> 128 -> 16 -> 2 level-1 betas. Host combines the 8 level-1 betas per tree
with the tiny root step.

Key algebraic restructurings (device does all O(N) work):
  - Leaf betas depend only on (position l, symbol s): they collapse into a
    256-row table; the level-6 einsum folds into T6[(l,s),(i,g)] so the whole
    leaf level becomes one-hot(symbol) matmuls.
  - Leaf log-nu contributions become histogram-counts x log-table (counts fall
    out of the one-hot generation for free via accum_out).
  - Interior levels: blocked matmuls with block-diagonal-over-g weights
    W_l[(j,g),(i,g)]; per-node normalize via sel/broadcast matmuls; log-nu via
    ScalarE Ln with free accumulation.
Partition packing everywhere: p = i*16 + g  (i = hidden state, g = generator).
"""
import os
import sys
import tempfile

import numpy as np

if '/opt/trn_rl_repo' not in sys.path:
    sys.path.insert(0, '/opt/trn_rl_repo')

import ml_dtypes

BF16 = ml_dtypes.bfloat16

K, DEPTH, NTREE, C, MSYM, NGEN = 8, 6, 2, 8, 32, 16
STARTS = [(K ** d - 1) // (K - 1) for d in range(DEPTH + 2)]
NT = STARTS[DEPTH + 1]          # 299593 nodes per tree
CG = C * NGEN                   # 128
NQ = 4                          # quarters per tree
LEAVES_Q = (K ** DEPTH) // NQ   # 65536 leaves per core
XP_LEN = 8192 + 1024 + 128 + 16 + 2   # interior-node symbols per core
XP_PAD = 9376
# per-level (parents U, chunks, xp offset)
LEVELS = [
    (8192, 16, 0),
    (1024, 2, 8192),
    (128, 1, 9216),
    (16, 1, 9344),
    (2, 1, 9360),
]
N_LL_SLOTS = 16 + 2 + 1 + 1 + 1 + 1   # per-chunk ll partials + leaf slot


def _softmax64(x, axis):
    x = np.asarray(x, np.float64)
    e = np.exp(x - x.max(axis=axis, keepdims=True))
    return e / e.sum(axis=axis, keepdims=True)


def _build_tables(A, B, Pi, SP):
    """Small O(params) tables, f64 on host. Returns dict of np arrays."""
    smA = _softmax64(A, 0)            # (C,C,K,G) over parent state i
    smB = _softmax64(B, 1)            # (C,M,G) over symbols
    smPi = _softmax64(Pi, 0)          # (C,K,G)
    smSP = _softmax64(SP, 0)          # (K,G)
    Mmat = smSP[:, None, None, :] * np.transpose(smA, (2, 0, 1, 3))  # [l,i,j,g]
    pb = smPi[:, :, None, :] * smB[:, None, :, :]     # (j, l, s, g)
    nuL = pb.sum(0)                                    # (l, s, g)
    betaLeaf = pb / nuL[None]
    llLeaf = np.log(nuL)                               # (l, s, g)
    T6 = np.einsum('lijg,jlsg->lsig', Mmat, betaLeaf)  # (l,s,i,g)
    T6f = T6.reshape(K * MSYM, CG)                     # rows (l,s), cols (i,g)
    Wl = np.zeros((K, CG, CG))
    ii = np.arange(C)
    for l in range(K):
        for g in range(NGEN):
            Wl[l, ii[:, None] * NGEN + g, ii[None, :] * NGEN + g] = Mmat[l, :, :, g].T
    BT = np.transpose(smB, (1, 0, 2)).reshape(MSYM, CG)
    llLeaf_f = llLeaf.reshape(K * MSYM, NGEN)

    p = np.arange(CG)
    sel = (p[:, None] % NGEN == np.arange(NGEN)[None, :]).astype(np.float64)
    E16 = sel.T.copy()
    return {
        'T6a': T6f[:128].astype(BF16),
        'T6b': T6f[128:].astype(BF16),
        'Wt': np.concatenate([Wl[l] for l in range(K)], axis=1).astype(BF16),  # [128, 1024]
        'BTt': BT.astype(BF16),                       # [32, 128]
        'selt': sel.astype(BF16),                     # [128, 16]
        'E16t': E16.astype(BF16),                     # [16, 128]
        'E16x4': np.concatenate([np.vstack([E16, np.zeros((16, CG))]) for _ in range(4)], axis=0).astype(BF16),  # [128, 128]
        'svec': (np.arange(128) % MSYM).reshape(128, 1).astype(np.float32),
        'svec32': np.arange(32).reshape(32, 1).astype(np.float32),
        'llLA': llLeaf_f[:128].astype(np.float32),    # [128, 16]
        'llLB': llLeaf_f[128:].astype(np.float32),    # [128, 16]
    }, Mmat.astype(np.float32), np.asarray(smB, np.float32)


def _build_bass():
    import concourse.bass as bass
    import concourse.bacc as bacc
    import concourse.mybir as mybir
    from concourse import tile

    f32 = mybir.dt.float32
    bf16 = mybir.dt.bfloat16
    Alu = mybir.AluOpType
    Act = mybir.ActivationFunctionType

    nc = bacc.Bacc(None, target_bir_lowering=False)

    # inputs
    u8 = mybir.dt.uint8
    xs_d = nc.dram_tensor('xs', [K, LEAVES_Q // K], u8, kind='ExternalInput')
    xp_d = nc.dram_tensor('xp', [1, XP_PAD], u8, kind='ExternalInput')
    tab_specs = [
        ('svec', [128, 1], f32), ('svec32', [32, 1], f32),
        ('T6a', [128, 128], bf16), ('T6b', [128, 128], bf16),
        ('BTt', [32, 128], bf16),
        ('selt', [128, 16], bf16), ('E16t', [16, 128], bf16),
        ('E16x4', [128, 128], bf16), ('Wt', [128, 1024], bf16),
        ('llLA', [128, 16], f32), ('llLB', [128, 16], f32),
    ]
    tab_d = {n: nc.dram_tensor(n, s, d, kind='ExternalInput') for n, s, d in tab_specs}
    beta1_d = nc.dram_tensor('beta1', [128, 2], f32, kind='ExternalOutput')
    llsum_d = nc.dram_tensor('llsum', [16, 1], f32, kind='ExternalOutput')

    with tile.TileContext(nc) as tc:
        with (
            tc.tile_pool(name='const', bufs=1) as constp,
            tc.tile_pool(name='beta', bufs=1) as betap,
            tc.tile_pool(name='oh', bufs=3) as ohp,
            tc.tile_pool(name='ohp32', bufs=3) as ohp32p,
            tc.tile_pool(name='bl', bufs=6) as blp,
            tc.tile_pool(name='bxs', bufs=4) as bxsbp,
            tc.tile_pool(name='rr', bufs=3) as rrp,
            tc.tile_pool(name='bxs2', bufs=3) as bxs2p,
            tc.tile_pool(name='lnout', bufs=3) as lnp,
            tc.tile_pool(name='acc', bufs=1) as accp,
            tc.tile_pool(name='cntp', bufs=8) as cntp,
            tc.tile_pool(name='xsb', bufs=6) as xsbp,
            tc.tile_pool(name='ps_tb', bufs=2, space='PSUM') as ps_tb,
            tc.tile_pool(name='ps_bx', bufs=2, space='PSUM') as ps_bx,
            tc.tile_pool(name='ps_nu', bufs=2, space='PSUM') as ps_nu,
            tc.tile_pool(name='ps_rb', bufs=2, space='PSUM') as ps_rb,
        ):
            # load constant tables
            tab = {}
            for n, s, d in tab_specs:
                t = constp.tile(s, d, tag=n)
                nc.sync.dma_start(t[:], tab_d[n][:])
                tab[n] = t

            xpb_all = accp.tile([32, XP_PAD], u8, name='xpb_all', tag='xpb_all')

            beta_bufs = [
                betap.tile([128, 8192], bf16, name='b5', tag='b5'),
                betap.tile([128, 1024], bf16, name='b4', tag='b4'),
                betap.tile([128, 128], bf16, name='b3', tag='b3'),
                betap.tile([128, 16], bf16, name='b2', tag='b2'),
                betap.tile([128, 2], f32, name='b1', tag='b1'),
            ]
            llparts = accp.tile([16, N_LL_SLOTS], f32, name='llparts', tag='llparts')
            cnts = accp.tile([128, 8], f32, name='cnts', tag='cnts')
            llsum_sb = accp.tile([16, 1], f32, name='llsum', tag='llsum')
            cA = accp.tile([128, 1], f32, name='cA', tag='cA')
            cB = accp.tile([128, 1], f32, name='cB', tag='cB')

            slot = 0
            cnt_tiles = []
            bl_pend = []
            GRP = 4      # chunks per one-hot batch at level 6
            for lev, (U, nch, xpo) in enumerate(LEVELS):
                N = U // nch
                out_beta = beta_bufs[lev]
                oh_grp = {}
                for c in range(nch):
                    tb_ps = ps_tb.tile([128, N], f32, name='tb', tag='tb')
                    # Bx = B[:, x_parent] via one-hot matmul
                    if c % GRP == 0:
                        NPW = min(N * GRP, U - (c // GRP) * N * GRP)
                        po = xpo + (c // GRP) * N * GRP
                        xp_bcast = bass.AP(xp_d[:].tensor, po, [[0, 32], [1, NPW]])
                        nc.sync.dma_start(xpb_all[:, po: po + NPW], xp_bcast)
                        ohP_g = ohp32p.tile([32, NPW], bf16, name='ohP', tag='ohP')
                        sv = tab['svec32'][:]
                        svb = bass.AP(sv.tensor, sv.offset, [list(sv.ap[0]), [0, NPW]])
                        nc.vector.tensor_tensor(ohP_g[:], xpb_all[:, po: po + NPW],
                                                svb, Alu.is_equal)
                        oh_grp['P'] = ohP_g
                    po2 = (c % GRP) * N
                    bx_ps = ps_bx.tile([128, N], f32, name='bx', tag='bx')
                    nc.tensor.matmul(bx_ps[:], tab['BTt'][:], oh_grp['P'][:, po2:po2 + N], start=True, stop=True)
                    bx_sb = bxsbp.tile([128, N], bf16, name='bxs', tag='bxs')
                    nc.scalar.copy(bx_sb[:], bx_ps[:])
                    if lev == 0:
                        if c % GRP == 0:
                            g4 = c // GRP
                            NW = N * GRP
                            ohA_g = ohp.tile([128, NW], bf16, name='ohA', tag='ohA')
                            ohB_g = ohp.tile([128, NW], bf16, name='ohB', tag='ohB')
                            for goff, oh_t, cslot in ((0, ohA_g, g4), (4, ohB_g, 4 + g4)):
                                xsb_t = xsbp.tile([128, NW], u8, name='xsb', tag='xsb')
                                src_ap = bass.AP(xs_d[:].tensor, goff * (LEAVES_Q // K) + g4 * NW,
                                                 [[LEAVES_Q // K, 4], [0, 32], [1, NW]])
                                nc.sync.dma_start(xsb_t[:], src_ap)
                                cntc = cntp.tile([128, 1], f32, name='cntc', tag='cntc')
                                nc.vector.tensor_scalar(
                                    oh_t[:], xsb_t[:], tab['svec'][:], None,
                                    Alu.is_equal, Alu.add,
                                    accum_out=cntc[:])
                                nc.scalar.copy(cnts[:, cslot:cslot + 1], cntc[:])
                            oh_grp.update({'A': ohA_g, 'B': ohB_g})
                        co = (c % GRP) * N
                        nc.tensor.matmul(tb_ps[:], tab['T6a'][:], oh_grp['A'][:, co:co + N], start=True, stop=False)
                        nc.tensor.matmul(tb_ps[:], tab['T6b'][:], oh_grp['B'][:, co:co + N], start=False, stop=True)
                    else:
                        child = beta_bufs[lev - 1]
                        bview = child[:].rearrange('p (u l) -> p u l', l=K)
                        for l in range(K):
                            nc.tensor.matmul(
                                tb_ps[:], tab['Wt'][:, 128 * l:128 * (l + 1)],
                                bview[:, c * N:(c + 1) * N, l],
                                start=(l == 0), stop=(l == K - 1))
                    # bl, nu, ll, normalize; nu for 4 chunks packs into one PSUM
                    # tile at 32-partition offsets so one reciprocal serves all 4
                    bl_t = blp.tile([128, N], bf16, name='bl', tag='bl')
                    nc.vector.tensor_mul(bl_t[:], tb_ps[:], bx_sb[:])
                    if c % GRP == 0:
                        nu4_ps = ps_nu.tile([128, N], f32, name='nu4', tag='nu4')
                        nc.vector.memset(nu4_ps[:], 1.0)
                    poff = 32 * (c % GRP)
                    nu_sl = nu4_ps[poff:poff + 16, :]
                    nc.tensor.matmul(nu_sl, tab['selt'][:], bl_t[:], start=True, stop=True,
                                     tile_position=(0, poff))
                    ln_t = lnp.tile([16, N], f32, name='ln', tag='ln')
                    nc.scalar.activation(ln_t[:], nu_sl, Act.Ln,
                                         accum_out=llparts[:, slot:slot + 1])
                    bl_pend.append((bl_t, c, poff))
                    if c % GRP == GRP - 1 or c == nch - 1:
                        r4_t = rrp.tile([128, N], bf16, name='r4', tag='r4')
                        with nc.allow_low_precision(reason='bf16 recip broadcast validated in numpy'):
                            nc.vector.reciprocal(r4_t[:], nu4_ps[:])
                        for bl_p, cp, poffp in bl_pend:
                            rb_ps = ps_rb.tile([128, N], f32, name='rb', tag='rb')
                            nc.tensor.matmul(rb_ps[:], tab['E16x4'][poffp:poffp + 16, :],
                                             r4_t[poffp:poffp + 16, :], start=True, stop=True,
                                             tile_position=(poffp, 0))
                            nc.vector.tensor_mul(out_beta[:, cp * N:(cp + 1) * N], bl_p[:], rb_ps[:])
                        bl_pend = []
                    slot += 1

            # leaf ll from histogram counts
            nc.vector.reduce_sum(cA[:], cnts[:, 0:4], axis=mybir.AxisListType.X)
            nc.vector.reduce_sum(cB[:], cnts[:, 4:8], axis=mybir.AxisListType.X)
            llf_ps = ps_nu.tile([16, 1], f32, name='llf', tag='nu4')
            nc.tensor.matmul(llf_ps[:], tab['llLA'][:], cA[:], start=True, stop=False)
            nc.tensor.matmul(llf_ps[:], tab['llLB'][:], cB[:], start=False, stop=True)
            nc.scalar.copy(llparts[:, slot:slot + 1], llf_ps[:])

            nc.vector.reduce_sum(llsum_sb[:], llparts[:], axis=mybir.AxisListType.X)
            nc.sync.dma_start(llsum_d[:], llsum_sb[:])
            nc.sync.dma_start(beta1_d[:], beta_bufs[4][:])
    if not nc.is_finalized():
        nc.finalize()
    return nc


_BASS_CACHE = {}


def _get_bass():
    if 'nc' not in _BASS_CACHE:
        _BASS_CACHE['nc'] = _build_bass()
    return _BASS_CACHE['nc']


def kernel(**inputs):
    from concourse.bass_utils import run_bass_kernel_spmd

    A = np.asarray(inputs['A']); B = np.asarray(inputs['B'])
    Pi = np.asarray(inputs['Pi']); SP = np.asarray(inputs['SP'])
    x = np.asarray(inputs['x'])

    tables, Mmat, smB = _build_tables(A, B, Pi, SP)

    in_maps = []
    for t in range(NTREE):
        base = t * NT
        for q in range(NQ):
            s6 = base + STARTS[6] + q * LEAVES_Q
            xs = x[s6: s6 + LEAVES_Q].astype(np.uint8)
            xs_t = np.ascontiguousarray(xs.reshape(LEAVES_Q // K, K).T)  # [8, 8192]
            xp = np.zeros((1, XP_PAD), np.uint8)
            off = 0
            for d in range(5, 0, -1):
                n_d = K ** d
                s = base + STARTS[d] + q * (n_d // NQ)
                xp[0, off: off + n_d // NQ] = x[s: s + n_d // NQ].astype(np.uint8)
                off += n_d // NQ
            m = {'xs': xs_t, 'xp': xp}
            m.update(tables)
            in_maps.append(m)

    nc = _get_bass()
    global _LAST_IN_MAPS
    _LAST_IN_MAPS = in_maps
    res = run_bass_kernel_spmd(nc, in_maps, core_ids=list(range(8)))
    results = res.results

    out = np.zeros((NTREE, NGEN), np.float32)
    for t in range(NTREE):
        beta1 = np.zeros((K, C, NGEN), np.float32)
        for q in range(NQ):
            r = results[t * NQ + q]
            out[t] += r['llsum'][:, 0]
            # beta1 dram [128, 2]: column n is quarter-node n, packing p=i*16+g
            beta1[2 * q: 2 * q + 2] = r['beta1'].T.reshape(2, C, NGEN)
        tb = np.einsum('lijg,ljg->ig', Mmat, beta1)
        bl = tb * smB[:, x[t * NT]]
        nu = bl.sum(0)
        out[t] += np.log(nu).astype(np.float32)
    return out

